# revision 30
# baseline (speedup 1.0000x reference)
"""Mesa-layer memory kernel for Trainium2 (8 NeuronCores, data-parallel over B).

Math: the reference's T-step Sherman-Morrison / discounted-accumulation
recurrence has a closed form,
    R_final = (I + K^T K)^{-1}            (eps term is O(1e-6) relative)
    S_final^T = K^T diag(c) V,   c_t = prod_{s>t} gamma_s
so per memory b the output is out_b = Q_b @ (R_b @ S_b^T).
R is computed with Newton-Schulz iterations in residual form
    X <- X + X^T (I - A X)
(bf16 iterations + one fp32 refinement; A = I + K^T K has cond ~3, so one
refinement lands at ~1e-5, far below the bf16 readout floor of ~3e-3).

v4 architecture — DMA-saturation pipeline. The kernel is HBM-bound:
33.6 MB/core at the measured ~425 GB/s per-core fabric rate = ~79 us
floor, so the design keeps the DMA queues streaming end to end and gives
every compute engine slack against the ~9.5 us/memory arrival cadence:
  * Queries are sharded host-side in TRANSPOSED layout [DK, NQ] (pure
    layout choice, same bytes moved), so the readout's Q^T operand loads
    directly: no PE transposes, no transpose-PSUM copies, and the
    readout chain is just cast -> 16 matmuls -> 4 copies -> store.
  * sync queue carries ALL input loads, interleaved K0 V0 K1 V1 K2 V2
    Q0 K3 V3 Q1 ... so each memory's K/V lead its Q by two memories;
    the scalar queue carries the 8 output stores, streaming from ~20 us.
  * V is cast to bf16 FIRST, then scaled in place by bf16(c) on DVE
    (bf16 multiply runs the fast 16-bit path; the fp32-width multiply
    measured 2.3-8 us under SBUF contention, bf16 ~0.6 us).
  * A and S^T accumulate in two separate single-bank PSUM tiles (each
    [P,512] fp32 = exactly one 2 KB zero region, so the two interleaved
    accumulation groups can never zero each other).
  * Casts alternate Scalar/DVE by memory parity so each engine's
    in-order stream follows the data-arrival order.
  * Emission is woven: each pair-group's NS iterations interleave with
    the next memories' accumulation matmuls and earlier memories'
    readout chunks.

Layout: timestep t maps to (partition p, slot r) via t = 16 p + r; every
HBM transfer is 8 KB/partition contiguous. The suffix cumprod of gammas
runs in log space: 16-step free-dim scans + one triangular matmul for the
cross-partition prefix.

Each core owns B/8 = 8 independent memories; no cross-core communication.
"""

import numpy as np

B, T, DK, DV, NQ = 64, 2048, 128, 128, 2048
NCORES = 8
BPC = B // NCORES          # memories per core
P = 128                    # partitions
R16 = T // P               # 16 row-slots per partition
GCLAMP = 1e-30             # gamma clamp before log (exact-0 gammas)

NS_BF = 3                  # Newton-Schulz iterations in bf16
NS_FP = 1                  # fp32 refinement iterations
# Optimal scalar NS seed X0 = s0*I: for A = I + K^T K with Gaussian K,
# lambda_min+lambda_max concentrates at 2*(1 + T + DK), so s0 = 1/(1+T+DK)
# gives residual ~0.47 -> 0.22 -> 0.049 -> 0.0024 in three iterations.
S0 = 1.0 / (1 + T + DK)
NGRP = 4                   # NS pair-groups
GSZ = BPC // NGRP          # 2 memories per group


def build_nc(ns_bf=NS_BF, ns_fp=NS_FP):
    import itertools

    import concourse.mybir as mybir
    import concourse.tile as tile
    from concourse import bacc
    from concourse.masks import make_identity, make_upper_triangular

    fp32 = mybir.dt.float32
    bf16 = mybir.dt.bfloat16
    AF = mybir.ActivationFunctionType
    OP = mybir.AluOpType
    AX = mybir.AxisListType
    NIT = ns_bf + ns_fp

    nc = bacc.Bacc(trn_type="TRN2", target_bir_lowering=False, debug=False)
    keys = nc.dram_tensor("keys", [BPC, T, DK], fp32, kind="ExternalInput").ap()
    values = nc.dram_tensor("values", [BPC, T, DV], fp32, kind="ExternalInput").ap()
    gammas = nc.dram_tensor("gammas", [BPC, T], fp32, kind="ExternalInput").ap()
    # host-transposed query layout: [DK, NQ] per memory
    queriesT = nc.dram_tensor("queriesT", [BPC, DK, NQ], fp32, kind="ExternalInput").ap()
    # blocked output layout: out_dev[i, m, s, v] = out[i, s*128 + m, v]
    # (the host un-blocks it; pure layout transform)
    out = nc.dram_tensor("out", [BPC, P, R16, DV], fp32, kind="ExternalOutput").ap()

    with tile.TileContext(nc) as tc:
        const = tc.alloc_tile_pool(name="const", bufs=1)
        gam = tc.alloc_tile_pool(name="gam", bufs=1)
        kp = tc.alloc_tile_pool(name="kp", bufs=4)
        vp = tc.alloc_tile_pool(name="vp", bufs=4)
        kbp = tc.alloc_tile_pool(name="kbp", bufs=3)
        vcbp = tc.alloc_tile_pool(name="vcbp", bufs=3)
        qp = tc.alloc_tile_pool(name="qp", bufs=5)
        qbp = tc.alloc_tile_pool(name="qbp", bufs=3)
        outp = tc.alloc_tile_pool(name="outp", bufs=3)
        small = tc.alloc_tile_pool(name="small", bufs=1)
        xs = tc.alloc_tile_pool(name="xs", bufs=2)
        ps_a = tc.alloc_tile_pool(name="ps_a", bufs=1, space="PSUM")
        ps_s = tc.alloc_tile_pool(name="ps_s", bufs=1, space="PSUM")
        ps_w = tc.alloc_tile_pool(name="ps_w", bufs=4, space="PSUM")
        ps_ro = tc.alloc_tile_pool(name="ps_ro", bufs=2, space="PSUM")

        ident = const.tile([P, P], fp32)
        make_identity(nc, ident)
        # identity pair for the group-batched I - A@X residual
        ident2 = const.tile([P, GSZ * P], fp32)
        for i in range(GSZ):
            make_identity(nc, ident2[:, i * P : (i + 1) * P])
        # strict upper triangular and all-ones for the cross-partition
        # prefix-sum of per-partition gamma-log totals
        utri = const.tile([P, P], fp32)
        make_upper_triangular(nc, utri, val=1.0, diag=False)
        ones2 = const.tile([P, P], fp32)
        nc.gpsimd.memset(ones2[:], 1.0)

        # ---- phase 0: suffix cumprod of gammas (log space) ----
        # g16[p, i, r] = gamma[i, 16p + r]
        g16 = gam.tile([P, BPC, R16], fp32)
        nc.sync.dma_start(g16[:], gammas.rearrange("i (p r) -> p i r", r=R16))
        g16f = g16.rearrange("p i r -> p (i r)")
        nc.vector.tensor_scalar_max(g16f, g16f, GCLAMP)
        nc.scalar.activation(g16f, g16f, AF.Ln)
        incl = gam.tile([P, BPC, R16], fp32)
        zz = gam.tile([P, R16], fp32)
        nc.vector.memset(zz[:], 0.0)
        # joiner: make DVE observe the ACT (Ln) dependency before the scans
        joiner = gam.tile([P, 1], fp32)
        nc.vector.tensor_copy(out=joiner[:], in_=g16[:, 0, 0:1])
        for i in range(BPC):
            nc.vector.tensor_tensor_scan(
                incl[:, i, :], g16[:, i, :], zz[:], 0.0, OP.add, OP.add
            )
        # per-partition totals -> cross-partition exclusive prefix + full sum
        ptot = gam.tile([P, BPC], fp32)
        nc.vector.tensor_copy(out=ptot[:], in_=incl[:, :, R16 - 1])
        ps_pre = ps_w.tile([P, 2 * BPC], fp32, tag="w", name="ps_pre")
        nc.tensor.matmul(ps_pre[:, 0:BPC], utri[:], ptot[:])          # offs
        nc.tensor.matmul(ps_pre[:, BPC : 2 * BPC], ones2[:], ptot[:])  # total
        pre_sb = gam.tile([P, 2 * BPC], fp32)
        nc.vector.tensor_copy(out=pre_sb[:], in_=ps_pre[:])
        bias2 = gam.tile([P, BPC], fp32)
        nc.vector.tensor_tensor(
            bias2[:], pre_sb[:, BPC : 2 * BPC], pre_sb[:, 0:BPC], OP.subtract
        )
        # c_t[p, i, r] = exp(bias - incl) = prod_{s > 16p+r} gamma[i, s]
        c_t = gam.tile([P, BPC, R16], fp32)
        for i in range(BPC):
            nc.scalar.activation(
                c_t[:, i, :], incl[:, i, :], AF.Exp,
                bias=bias2[:, i : i + 1], scale=-1.0,
            )
        # bf16 copy of c for the 16-bit fast-path multiply
        c_bf = gam.tile([P, BPC, R16], bf16)
        nc.vector.tensor_copy(out=c_bf[:], in_=c_t[:])

        # ---- load emission: ALL inputs on the sync queue ----
        k_sb = [None] * BPC
        v_sb = [None] * BPC
        q_sb = [None] * BPC
        kb = [None] * BPC
        vcb = [None] * BPC
        qtb = [None] * BPC

        def load_k(i):
            k_sb[i] = kp.tile([P, R16, DK], fp32, tag="k", name=f"k{i}")
            nc.sync.dma_start(
                k_sb[i][:], keys[i].rearrange("(p r) k -> p r k", p=P)
            )

        def load_v(i):
            v_sb[i] = vp.tile([P, R16, DV], fp32, tag="v", name=f"v{i}")
            nc.sync.dma_start(
                v_sb[i][:], values[i].rearrange("(p r) k -> p r k", p=P)
            )

        def load_q(i):
            q_sb[i] = qp.tile([P, NQ], fp32, tag="q", name=f"q{i}")
            nc.sync.dma_start(q_sb[i][:], queriesT[i])

        # sync queue: K0 V0 K1 V1 K2 V2 Q0 K3 V3 Q1 ... K7 V7 Q5 Q6 Q7
        load_k(0); load_v(0)
        load_k(1); load_v(1)
        load_k(2); load_v(2)
        load_q(0)
        for i in range(3, BPC):
            load_k(i); load_v(i)
            load_q(i - 3)
        load_q(5); load_q(6); load_q(7)

        # ---- per-memory state tiles ----
        A_sb = [small.tile([P, P], fp32, tag=f"A{i}", name=f"A{i}") for i in range(BPC)]
        A_bf = [small.tile([P, P], bf16, tag=f"Ab{i}", name=f"Ab{i}") for i in range(BPC)]
        ST_sb = [small.tile([P, P], fp32, tag=f"S{i}", name=f"S{i}") for i in range(BPC)]
        Phi_bf = [small.tile([P, P], bf16, tag=f"Pb{i}", name=f"Phib{i}") for i in range(BPC)]
        Xg = [None] * NGRP

        def prep(i):
            """K/V casts (Scalar/DVE by parity), bf16 c-scaling, A/S accum.
            All big ops are chopped into sub-tile chunks so the in-order
            engine streams never block an NS chain hop for long."""
            kb[i] = kbp.tile([P, R16, DK], bf16, tag="kb", name=f"kb{i}")
            vcb[i] = vcbp.tile([P, R16, DV], bf16, tag="vcb", name=f"vcb{i}")
            h = R16 // 2
            keng, veng = ((nc.scalar.copy, nc.vector.tensor_copy)
                          if i % 2 == 0 else
                          (nc.vector.tensor_copy, nc.scalar.copy))
            for lo, hi in ((0, h), (h, R16)):
                keng(out=kb[i][:, lo:hi, :], in_=k_sb[i][:, lo:hi, :])
                veng(out=vcb[i][:, lo:hi, :], in_=v_sb[i][:, lo:hi, :])
                yield
            for cc in range(4):
                sl = slice(4 * cc, 4 * cc + 4)
                nc.vector.tensor_tensor(
                    vcb[i][:, sl, :], vcb[i][:, sl, :],
                    c_bf[:, i, sl, None].to_broadcast((P, 4, DV)),
                    OP.mult,
                )
                yield
            psa = ps_a.tile([P, 512], fp32, tag="a", name=f"psa{i}")
            pss = ps_s.tile([P, 512], fp32, tag="s", name=f"pss{i}")
            for r in range(R16):
                nc.tensor.matmul(
                    psa[:, 0:P], kb[i][:, r, :], kb[i][:, r, :],
                    start=(r == 0), stop=(r == R16 - 1),
                )
                nc.tensor.matmul(
                    pss[:, 0:P], kb[i][:, r, :], vcb[i][:, r, :],
                    start=(r == 0), stop=(r == R16 - 1),
                )
                if r % 2 == 1:
                    yield
            nc.vector.tensor_tensor(A_sb[i][:], psa[:, 0:P], ident[:], OP.add)
            nc.vector.tensor_copy(out=ST_sb[i][:], in_=pss[:, 0:P])
            nc.scalar.copy(out=A_bf[i][:], in_=A_sb[i][:])
            yield

        def x0(g):
            xw = xs.tile([P, GSZ * P], bf16, tag=f"Xb{g}", name=f"Xb{g}_0")
            for i in range(GSZ):
                nc.scalar.activation(
                    xw[:, i * P : (i + 1) * P], ident[:], AF.Copy, scale=S0,
                )
            Xg[g] = xw

        def ns_group(g):
            """All NS iterations for pair-group g, yielding between stages."""
            for it in range(NIT):
                bf_iter = it < ns_bf
                last_bf = it == ns_bf - 1
                Amat = A_bf if bf_iter else A_sb
                pa = ps_w.tile([P, GSZ * P], fp32, tag="w", name=f"pa{g}_{it}")
                for i in range(GSZ):
                    sl = slice(i * P, (i + 1) * P)
                    nc.tensor.matmul(pa[:, sl], Amat[GSZ * g + i][:], Xg[g][:, sl])
                yield
                eg = xs.tile(
                    [P, GSZ * P], bf16 if bf_iter else fp32,
                    tag=f"e_{bf_iter}", name=f"e{g}_{it}",
                )
                nc.vector.scalar_tensor_tensor(
                    eg[:], pa[:], -1.0, ident2[:], OP.mult, OP.add
                )
                yield
                pb = ps_w.tile([P, GSZ * P], fp32, tag="w", name=f"pb{g}_{it}")
                for i in range(GSZ):
                    sl = slice(i * P, (i + 1) * P)
                    nc.tensor.matmul(pb[:, sl], Xg[g][:, sl], eg[:, sl])
                yield
                out_fp32 = (not bf_iter) or last_bf
                xn = xs.tile(
                    [P, GSZ * P], fp32 if out_fp32 else bf16,
                    tag=f"Xf{g}" if out_fp32 else f"Xb{g}",
                    name=f"X{g}_{it + 1}",
                )
                nc.vector.tensor_tensor(xn[:], Xg[g][:], pb[:], OP.add)
                Xg[g] = xn
                yield
            for i in range(GSZ * g, GSZ * g + GSZ):
                phi(i)
            yield

        def phi(i):
            g, sl = i // GSZ, slice((i % GSZ) * P, (i % GSZ + 1) * P)
            ps_phi = ps_w.tile([P, P], fp32, tag="w", name=f"ps_phi{i}")
            nc.tensor.matmul(ps_phi[:], Xg[g][:, sl], ST_sb[i][:])
            nc.scalar.copy(out=Phi_bf[i][:], in_=ps_phi[:])

        def ro(i):
            """Readout: cast Q^T, 4x(4 matmuls + PSUM copy), store.
            psum block s covers query rows n in [s*128, (s+1)*128):
            o_sb[m, s, v] = out[i, s*128 + m, v]."""
            qtb[i] = qbp.tile([P, NQ], bf16, tag="qtb", name=f"qtb{i}")
            q4 = NQ // 4
            for cc in range(4):
                sl = slice(cc * q4, (cc + 1) * q4)
                if cc % 2 == 0:
                    nc.scalar.copy(out=qtb[i][:, sl], in_=q_sb[i][:, sl])
                else:
                    nc.vector.tensor_copy(out=qtb[i][:, sl], in_=q_sb[i][:, sl])
                yield
            o_sb = outp.tile([P, R16, DV], fp32, tag="o", name=f"o{i}")
            for c in range(4):
                ps_o = ps_ro.tile([P, 4 * P], fp32, tag="ro", name=f"ps_o{i}_{c}")
                for j in range(4):
                    s = 4 * c + j
                    nc.tensor.matmul(
                        ps_o[:, j * P : (j + 1) * P],
                        qtb[i][:, s * P : (s + 1) * P], Phi_bf[i][:],
                    )
                yield
                sl = slice(4 * c, 4 * c + 4)
                if c % 2 == 0:
                    nc.vector.tensor_copy(out=o_sb[:, sl, :], in_=ps_o[:])
                else:
                    nc.scalar.copy(out=o_sb[:, sl, :], in_=ps_o[:])
                yield
            nc.scalar.dma_start(out[i][:], o_sb[:])
            yield

        def weave(*gens):
            """Round-robin generators; (gen, w) advances w yields per round."""
            active = []
            for x in gens:
                g, w = x if isinstance(x, tuple) else (x, 1)
                active.append([iter(g), w])
            while active:
                for a in list(active):
                    try:
                        for _ in range(a[1]):
                            next(a[0])
                    except StopIteration:
                        active.remove(a)

        chain = itertools.chain

        # ---- woven emission ----
        for _ in prep(0):
            pass
        for _ in prep(1):
            pass
        x0(0)
        weave((ns_group(0), 3), chain(prep(2), prep(3)))
        x0(1)
        weave((ns_group(1), 3), (ro(0), 2), (ro(1), 2),
              chain(prep(4), prep(5)))
        x0(2)
        weave((ns_group(2), 3), (ro(2), 2), (ro(3), 2),
              chain(prep(6), prep(7)))
        x0(3)
        weave((ns_group(3), 3), (ro(4), 2), (ro(5), 2))
        weave(ro(6), ro(7))

        for pool in (ps_ro, ps_w, ps_s, ps_a, xs, small, outp, qbp, qp, vcbp,
                     kbp, vp, kp, gam, const):
            pool.release()

    if not nc.is_finalized():
        nc.finalize()
    return nc


def make_in_maps(inputs):
    """Shard full inputs across cores (host-side layout transforms only)."""
    keys = np.ascontiguousarray(inputs["keys"], dtype=np.float32)
    values = np.ascontiguousarray(inputs["values"], dtype=np.float32)
    gammas = np.ascontiguousarray(inputs["gammas"], dtype=np.float32)
    queries = np.ascontiguousarray(inputs["queries"], dtype=np.float32)
    queriesT = np.ascontiguousarray(queries.transpose(0, 2, 1))
    in_maps = []
    for m in range(NCORES):
        s = slice(m * BPC, (m + 1) * BPC)
        in_maps.append(
            {
                "keys": keys[s],
                "values": values[s],
                "gammas": gammas[s],
                "queriesT": queriesT[s],
            }
        )
    return in_maps


def assemble_out(results):
    """Gather per-core outputs; un-block out_dev[i, m, s, v] -> [i, n, v]."""
    out_dev = np.concatenate([results[m]["out"] for m in range(NCORES)], axis=0)
    return np.ascontiguousarray(out_dev.transpose(0, 2, 1, 3).reshape(B, NQ, DV))


def kernel(**inputs) -> np.ndarray:
    from concourse.bass_utils import run_bass_kernel_spmd

    nc = build_nc()
    res = run_bass_kernel_spmd(
        nc, make_in_maps(inputs), core_ids=list(range(NCORES))
    )
    return assemble_out(res.results)


# revision 36
# speedup vs baseline: 1.1926x; 1.1926x over previous
"""Mesa-layer memory kernel for Trainium2 (8 NeuronCores, data-parallel over B).

Math: the reference's T-step Sherman-Morrison / discounted-accumulation
recurrence has a closed form,
    R_final = (I + K^T K)^{-1}            (eps term is O(1e-6) relative)
    S_final^T = K^T diag(c) V,   c_t = prod_{s>t} gamma_s
so per memory b the output is out_b = Q_b @ (R_b @ S_b^T).
R is computed with Newton-Schulz iterations in residual form
    X <- X + X^T (I - A X)
(bf16 iterations + one fp32 refinement; A = I + K^T K has cond ~3, so one
refinement lands at ~1e-5, far below the bf16 readout floor of ~3e-3).

v4 architecture — DMA-saturation pipeline. The kernel is HBM-bound:
33.6 MB/core at the measured ~425 GB/s per-core fabric rate = ~79 us
floor, so the design keeps the DMA queues streaming end to end and gives
every compute engine slack against the ~9.5 us/memory arrival cadence:
  * Queries are sharded host-side in TRANSPOSED layout [DK, NQ] (pure
    layout choice, same bytes moved), so the readout's Q^T operand loads
    directly: no PE transposes, no transpose-PSUM copies, and the
    readout chain is just cast -> 16 matmuls -> 4 copies -> store.
  * sync queue carries ALL input loads, interleaved K0 V0 K1 V1 K2 V2
    Q0 K3 V3 Q1 ... so each memory's K/V lead its Q by two memories;
    the scalar queue carries the 8 output stores, streaming from ~20 us.
  * V is cast to bf16 FIRST, then scaled in place by bf16(c) on DVE
    (bf16 multiply runs the fast 16-bit path; the fp32-width multiply
    measured 2.3-8 us under SBUF contention, bf16 ~0.6 us).
  * A and S^T accumulate in two separate single-bank PSUM tiles (each
    [P,512] fp32 = exactly one 2 KB zero region, so the two interleaved
    accumulation groups can never zero each other).
  * Casts alternate Scalar/DVE by memory parity so each engine's
    in-order stream follows the data-arrival order.
  * Emission is woven: each pair-group's NS iterations interleave with
    the next memories' accumulation matmuls and earlier memories'
    readout chunks.

Layout: timestep t maps to (partition p, slot r) via t = 16 p + r; every
HBM transfer is 8 KB/partition contiguous. The suffix cumprod of gammas
runs in log space: 16-step free-dim scans + one triangular matmul for the
cross-partition prefix.

Each core owns B/8 = 8 independent memories; no cross-core communication.
"""

import numpy as np

B, T, DK, DV, NQ = 64, 2048, 128, 128, 2048
NCORES = 8
BPC = B // NCORES          # memories per core
P = 128                    # partitions
R16 = T // P               # 16 row-slots per partition
GCLAMP = 1e-30             # gamma clamp before log (exact-0 gammas)

NS_BF = 3                  # Newton-Schulz iterations in bf16
NS_FP = 1                  # fp32 refinement iterations
# Optimal scalar NS seed X0 = s0*I: for A = I + K^T K with Gaussian K,
# lambda_min+lambda_max concentrates at 2*(1 + T + DK), so s0 = 1/(1+T+DK)
# gives residual ~0.47 -> 0.22 -> 0.049 -> 0.0024 in three iterations.
S0 = 1.0 / (1 + T + DK)
NGRP = 4                   # NS pair-groups
GSZ = BPC // NGRP          # 2 memories per group


def build_nc(ns_bf=NS_BF, ns_fp=NS_FP):
    import itertools

    import concourse.mybir as mybir
    import concourse.tile as tile
    from concourse import bacc
    from concourse.masks import make_identity, make_upper_triangular

    fp32 = mybir.dt.float32
    bf16 = mybir.dt.bfloat16
    AF = mybir.ActivationFunctionType
    OP = mybir.AluOpType
    AX = mybir.AxisListType
    NIT = ns_bf + ns_fp

    nc = bacc.Bacc(trn_type="TRN2", target_bir_lowering=False, debug=False)
    keys = nc.dram_tensor("keys", [BPC, T, DK], fp32, kind="ExternalInput").ap()
    values = nc.dram_tensor("values", [BPC, T, DV], fp32, kind="ExternalInput").ap()
    gammas = nc.dram_tensor("gammas", [BPC, T], fp32, kind="ExternalInput").ap()
    # host-transposed query layout: [DK, NQ] per memory
    queriesT = nc.dram_tensor("queriesT", [BPC, DK, NQ], fp32, kind="ExternalInput").ap()
    # transposed output layout: out_dev[i, v, n] = out[i, n, v]
    # (the host un-transposes it; pure layout transform)
    out = nc.dram_tensor("out", [BPC, DV, NQ], fp32, kind="ExternalOutput").ap()

    with tile.TileContext(nc) as tc:
        const = tc.alloc_tile_pool(name="const", bufs=1)
        gam = tc.alloc_tile_pool(name="gam", bufs=1)
        kp = tc.alloc_tile_pool(name="kp", bufs=4)
        vp = tc.alloc_tile_pool(name="vp", bufs=4)
        kbp = tc.alloc_tile_pool(name="kbp", bufs=3)
        vcbp = tc.alloc_tile_pool(name="vcbp", bufs=3)
        qp = tc.alloc_tile_pool(name="qp", bufs=5)
        qbp = tc.alloc_tile_pool(name="qbp", bufs=3)
        outp = tc.alloc_tile_pool(name="outp", bufs=3)
        small = tc.alloc_tile_pool(name="small", bufs=1)
        xs = tc.alloc_tile_pool(name="xs", bufs=2)
        ps_a = tc.alloc_tile_pool(name="ps_a", bufs=1, space="PSUM")
        ps_s = tc.alloc_tile_pool(name="ps_s", bufs=1, space="PSUM")
        ps_w = tc.alloc_tile_pool(name="ps_w", bufs=4, space="PSUM")
        ps_ro = tc.alloc_tile_pool(name="ps_ro", bufs=2, space="PSUM")

        ident = const.tile([P, P], fp32)
        make_identity(nc, ident)
        # identity pair for the group-batched I - A@X residual
        ident2 = const.tile([P, GSZ * P], fp32)
        for i in range(GSZ):
            make_identity(nc, ident2[:, i * P : (i + 1) * P])
        # strict upper triangular and all-ones for the cross-partition
        # prefix-sum of per-partition gamma-log totals
        utri = const.tile([P, P], fp32)
        make_upper_triangular(nc, utri, val=1.0, diag=False)
        ones2 = const.tile([P, P], fp32)
        nc.gpsimd.memset(ones2[:], 1.0)

        # ---- phase 0: suffix cumprod of gammas (log space) ----
        # g16[p, i, r] = gamma[i, 16p + r]
        g16 = gam.tile([P, BPC, R16], fp32)
        nc.sync.dma_start(g16[:], gammas.rearrange("i (p r) -> p i r", r=R16))
        g16f = g16.rearrange("p i r -> p (i r)")
        nc.vector.tensor_scalar_max(g16f, g16f, GCLAMP)
        nc.scalar.activation(g16f, g16f, AF.Ln)
        incl = gam.tile([P, BPC, R16], fp32)
        zz = gam.tile([P, R16], fp32)
        nc.vector.memset(zz[:], 0.0)
        # joiner: make DVE observe the ACT (Ln) dependency before the scans
        joiner = gam.tile([P, 1], fp32)
        nc.vector.tensor_copy(out=joiner[:], in_=g16[:, 0, 0:1])
        for i in range(BPC):
            nc.vector.tensor_tensor_scan(
                incl[:, i, :], g16[:, i, :], zz[:], 0.0, OP.add, OP.add
            )
        # per-partition totals -> cross-partition exclusive prefix + full sum
        ptot = gam.tile([P, BPC], fp32)
        nc.vector.tensor_copy(out=ptot[:], in_=incl[:, :, R16 - 1])
        ps_pre = ps_w.tile([P, 2 * BPC], fp32, tag="w", name="ps_pre")
        nc.tensor.matmul(ps_pre[:, 0:BPC], utri[:], ptot[:])          # offs
        nc.tensor.matmul(ps_pre[:, BPC : 2 * BPC], ones2[:], ptot[:])  # total
        pre_sb = gam.tile([P, 2 * BPC], fp32)
        nc.vector.tensor_copy(out=pre_sb[:], in_=ps_pre[:])
        bias2 = gam.tile([P, BPC], fp32)
        nc.vector.tensor_tensor(
            bias2[:], pre_sb[:, BPC : 2 * BPC], pre_sb[:, 0:BPC], OP.subtract
        )
        # c_t[p, i, r] = exp(bias - incl) = prod_{s > 16p+r} gamma[i, s]
        c_t = gam.tile([P, BPC, R16], fp32)
        for i in range(BPC):
            nc.scalar.activation(
                c_t[:, i, :], incl[:, i, :], AF.Exp,
                bias=bias2[:, i : i + 1], scale=-1.0,
            )
        # bf16 copy of c for the 16-bit fast-path multiply
        c_bf = gam.tile([P, BPC, R16], bf16)
        nc.vector.tensor_copy(out=c_bf[:], in_=c_t[:])

        # ---- load emission: ALL inputs on the sync queue ----
        k_sb = [None] * BPC
        v_sb = [None] * BPC
        q_sb = [None] * BPC
        kb = [None] * BPC
        vcb = [None] * BPC
        qtb = [None] * BPC

        def load_k(i):
            k_sb[i] = kp.tile([P, R16, DK], fp32, tag="k", name=f"k{i}")
            nc.sync.dma_start(
                k_sb[i][:], keys[i].rearrange("(p r) k -> p r k", p=P)
            )

        def load_v(i):
            v_sb[i] = vp.tile([P, R16, DV], fp32, tag="v", name=f"v{i}")
            nc.sync.dma_start(
                v_sb[i][:], values[i].rearrange("(p r) k -> p r k", p=P)
            )

        def load_q(i):
            q_sb[i] = qp.tile([P, NQ], fp32, tag="q", name=f"q{i}")
            nc.sync.dma_start(q_sb[i][:], queriesT[i])

        # sync queue: K0 V0 K1 V1 K2 V2 Q0 K3 V3 Q1 ... K7 V7 Q5 Q6 Q7
        load_k(0); load_v(0)
        load_k(1); load_v(1)
        load_k(2); load_v(2)
        load_q(0)
        for i in range(3, BPC):
            load_k(i); load_v(i)
            load_q(i - 3)
        load_q(5); load_q(6); load_q(7)

        # ---- per-memory state tiles ----
        A_sb = [small.tile([P, P], fp32, tag=f"A{i}", name=f"A{i}") for i in range(BPC)]
        A_bf = [small.tile([P, P], bf16, tag=f"Ab{i}", name=f"Ab{i}") for i in range(BPC)]
        ST_bf = [small.tile([P, P], bf16, tag=f"S{i}", name=f"S{i}") for i in range(BPC)]
        Phi_bf = [small.tile([P, P], bf16, tag=f"Pb{i}", name=f"Phib{i}") for i in range(BPC)]
        Xg = [None] * NGRP

        def prep(i):
            """K/V casts (Scalar/DVE by parity), bf16 c-scaling, A/S accum.
            All big ops are chopped into sub-tile chunks so the in-order
            engine streams never block an NS chain hop for long."""
            kb[i] = kbp.tile([P, R16, DK], bf16, tag="kb", name=f"kb{i}")
            vcb[i] = vcbp.tile([P, R16, DV], bf16, tag="vcb", name=f"vcb{i}")
            h = R16 // 2
            keng, veng = ((nc.scalar.copy, nc.vector.tensor_copy)
                          if i % 2 == 0 else
                          (nc.vector.tensor_copy, nc.scalar.copy))
            for lo, hi in ((0, h), (h, R16)):
                keng(out=kb[i][:, lo:hi, :], in_=k_sb[i][:, lo:hi, :])
                veng(out=vcb[i][:, lo:hi, :], in_=v_sb[i][:, lo:hi, :])
                yield
            for cc in range(4):
                sl = slice(4 * cc, 4 * cc + 4)
                nc.vector.tensor_tensor(
                    vcb[i][:, sl, :], vcb[i][:, sl, :],
                    c_bf[:, i, sl, None].to_broadcast((P, 4, DV)),
                    OP.mult,
                )
                yield
            psa = ps_a.tile([P, 512], fp32, tag="a", name=f"psa{i}")
            pss = ps_s.tile([P, 512], fp32, tag="s", name=f"pss{i}")
            for r in range(R16):
                nc.tensor.matmul(
                    psa[:, 0:P], kb[i][:, r, :], kb[i][:, r, :],
                    start=(r == 0), stop=(r == R16 - 1),
                )
                nc.tensor.matmul(
                    pss[:, 0:P], kb[i][:, r, :], vcb[i][:, r, :],
                    start=(r == 0), stop=(r == R16 - 1),
                )
                if r % 2 == 1:
                    yield
            nc.vector.tensor_tensor(A_sb[i][:], psa[:, 0:P], ident[:], OP.add)
            nc.vector.tensor_copy(out=ST_bf[i][:], in_=pss[:, 0:P])
            nc.scalar.copy(out=A_bf[i][:], in_=A_sb[i][:])
            yield

        def x0(g):
            xw = xs.tile([P, GSZ * P], bf16, tag=f"Xb{g}", name=f"Xb{g}_0")
            for i in range(GSZ):
                nc.scalar.activation(
                    xw[:, i * P : (i + 1) * P], ident[:], AF.Copy, scale=S0,
                )
            Xg[g] = xw

        def ns_group(g):
            """All NS iterations for pair-group g, yielding between stages."""
            for it in range(NIT):
                bf_iter = it < ns_bf
                last_bf = it == ns_bf - 1
                Amat = A_bf if bf_iter else A_sb
                pa = ps_w.tile([P, GSZ * P], fp32, tag="w", name=f"pa{g}_{it}")
                for i in range(GSZ):
                    sl = slice(i * P, (i + 1) * P)
                    nc.tensor.matmul(pa[:, sl], Amat[GSZ * g + i][:], Xg[g][:, sl])
                yield
                eg = xs.tile(
                    [P, GSZ * P], bf16 if bf_iter else fp32,
                    tag=f"e_{bf_iter}", name=f"e{g}_{it}",
                )
                nc.vector.scalar_tensor_tensor(
                    eg[:], pa[:], -1.0, ident2[:], OP.mult, OP.add
                )
                yield
                pb = ps_w.tile([P, GSZ * P], fp32, tag="w", name=f"pb{g}_{it}")
                for i in range(GSZ):
                    sl = slice(i * P, (i + 1) * P)
                    nc.tensor.matmul(pb[:, sl], Xg[g][:, sl], eg[:, sl])
                yield
                out_fp32 = (not bf_iter) or last_bf
                xn = xs.tile(
                    [P, GSZ * P], fp32 if out_fp32 else bf16,
                    tag=f"Xf{g}" if out_fp32 else f"Xb{g}",
                    name=f"X{g}_{it + 1}",
                )
                nc.vector.tensor_tensor(xn[:], Xg[g][:], pb[:], OP.add)
                Xg[g] = xn
                yield
            xbf = xs.tile([P, GSZ * P], bf16, tag=f"Xc{g}", name=f"Xc{g}")
            nc.scalar.copy(out=xbf[:], in_=Xg[g][:])
            Xg[g] = xbf
            yield
            for i in range(GSZ * g, GSZ * g + GSZ):
                phi(i)
            yield

        def phi(i):
            g, sl = i // GSZ, slice((i % GSZ) * P, (i % GSZ + 1) * P)
            ps_phi = ps_w.tile([P, P], fp32, tag="w", name=f"ps_phi{i}")
            nc.tensor.matmul(ps_phi[:], Xg[g][:, sl], ST_bf[i][:])
            nc.scalar.copy(out=Phi_bf[i][:], in_=ps_phi[:])

        def ro(i):
            """Readout, transposed: Phi^T stationary, Q^T moving 512-wide.
            Per 512-query chunk: cast -> one matmul -> one PSUM copy.
            o_sbT[v, n] = out[i, n, v]; the host un-transposes."""
            qtb[i] = qbp.tile([P, NQ], bf16, tag="qtb", name=f"qtb{i}")
            o_sbT = outp.tile([P, NQ], fp32, tag="o", name=f"o{i}")
            q4 = NQ // 4
            for cc in range(4):
                sl = slice(cc * q4, (cc + 1) * q4)
                if cc % 2 == 0:
                    nc.scalar.copy(out=qtb[i][:, sl], in_=q_sb[i][:, sl])
                else:
                    nc.vector.tensor_copy(out=qtb[i][:, sl], in_=q_sb[i][:, sl])
                yield
                ps_o = ps_ro.tile([P, q4], fp32, tag="ro", name=f"ps_o{i}_{cc}")
                nc.tensor.matmul(ps_o[:], Phi_bf[i][:], qtb[i][:, sl])
                yield
                if cc % 2 == 0:
                    nc.vector.tensor_copy(out=o_sbT[:, sl], in_=ps_o[:])
                else:
                    nc.scalar.copy(out=o_sbT[:, sl], in_=ps_o[:])
                yield
            nc.scalar.dma_start(out[i][:], o_sbT[:])
            yield

        def weave(*gens):
            """Round-robin generators; (gen, w) advances w yields per round."""
            active = []
            for x in gens:
                g, w = x if isinstance(x, tuple) else (x, 1)
                active.append([iter(g), w])
            while active:
                for a in list(active):
                    try:
                        for _ in range(a[1]):
                            next(a[0])
                    except StopIteration:
                        active.remove(a)

        chain = itertools.chain

        # ---- woven emission ----
        for _ in prep(0):
            pass
        for _ in prep(1):
            pass
        x0(0)
        weave((ns_group(0), 3), chain(prep(2), prep(3)))
        x0(1)
        weave((ns_group(1), 3), (ro(0), 2), (ro(1), 2),
              chain(prep(4), prep(5)))
        x0(2)
        weave((ns_group(2), 3), (ro(2), 2), (ro(3), 2),
              chain(prep(6), prep(7)))
        x0(3)
        weave((ns_group(3), 3), (ro(4), 2), (ro(5), 2))
        weave(ro(6), ro(7))

        for pool in (ps_ro, ps_w, ps_s, ps_a, xs, small, outp, qbp, qp, vcbp,
                     kbp, vp, kp, gam, const):
            pool.release()

    if not nc.is_finalized():
        nc.finalize()
    return nc


def make_in_maps(inputs):
    """Shard full inputs across cores (host-side layout transforms only)."""
    keys = np.ascontiguousarray(inputs["keys"], dtype=np.float32)
    values = np.ascontiguousarray(inputs["values"], dtype=np.float32)
    gammas = np.ascontiguousarray(inputs["gammas"], dtype=np.float32)
    queries = np.ascontiguousarray(inputs["queries"], dtype=np.float32)
    queriesT = np.ascontiguousarray(queries.transpose(0, 2, 1))
    in_maps = []
    for m in range(NCORES):
        s = slice(m * BPC, (m + 1) * BPC)
        in_maps.append(
            {
                "keys": keys[s],
                "values": values[s],
                "gammas": gammas[s],
                "queriesT": queriesT[s],
            }
        )
    return in_maps


def assemble_out(results):
    """Gather per-core outputs; un-transpose out_dev[i, v, n] -> [i, n, v]."""
    out_dev = np.concatenate([results[m]["out"] for m in range(NCORES)], axis=0)
    return np.ascontiguousarray(out_dev.transpose(0, 2, 1))


def kernel(**inputs) -> np.ndarray:
    from concourse.bass_utils import run_bass_kernel_spmd

    nc = build_nc()
    res = run_bass_kernel_spmd(
        nc, make_in_maps(inputs), core_ids=list(range(NCORES))
    )
    return assemble_out(res.results)
